# revision 1
# baseline (speedup 1.0000x reference)
"""Trainium2 Bass kernel for nn_Decoder (mask-multiply + dense [512,16] + overlap-and-add).

Full-input contract: kernel(**inputs) takes the complete tensors, shards
batch-wise across 8 NeuronCores (2 batches per core, both speakers on-core),
runs one SPMD Bass program, and gathers the full [16, 2, 32696] output.

Per-core algorithm (b = 2 batches, frame = 4086, basis = 512, spk = 2, L = 16),
per block of 512 frames:
  1. One 3MB DMA loads host-concatenated [inputs | estmask] -> SBUF
     cat[128, 4, 1536] (f on partitions), alternating the SP/ACT HWDGE rings.
  2. DVE (+1 GPSIMD subtile): de-interleave + mask-multiply
     -> xx[128, 4, 1024] (free dim = s*512 + c)
  3. PE transpose 128x128 blocks -> PSUM -> ACT copy -> xxT[128, 8, 512]
     (c2 = s*512+c on partitions, f on free dim)
  4. PE matmul per speaker: yyT[16, Fb] += W[ck].T @ xxT[ck]  (4 c-chunks)
  5. ACT copy yyT -> SBUF staging st[16, Fb]; gpsimd SBUF->SBUF DMA shifts the
     high taps st[8:16] into row buffer zb[8, 4087] at column f0+1 (DMA is the
     only engine free of partition-base alignment constraints)
  6. DVE overlap-add: z[j, k] = st[j, k] + zb[j, k]  (zb col 0 zero)
  7. PE-transpose z -> [128, 4, 8] so the store writes 32B-contiguous runs
     (a j-on-partition store emits one 4B descriptor per element - 10x slower);
     one tail column k = 4086 comes straight from zb after the last block.

The build also post-processes the scheduled program with _split_excess_waits:
this container's walrus rejects any instruction carrying more than one
semaphore wait.
"""

import sys

for _p in ("/opt/trn_rl_repo", "/root/.axon_site/_ro/trn_rl_repo"):
    if _p not in sys.path:
        sys.path.append(_p)

import numpy as np

# Problem constants (hardcoded per contract; kernel.py may not read spec.json).
BS = 16
FRAME = 4086
BASIS = 512
SPK = 2
L = 16
STEP = L // 2
OUT_LEN = (FRAME - 1) * STEP + L  # 32696
NSEG = OUT_LEN // STEP  # 4087 == FRAME + 1
N_CORES = 8
B_PER_CORE = BS // N_CORES  # 2


def _split_excess_waits(nc, max_waits=1):
    """This toolchain's walrus rejects >1 semaphore wait per instruction
    ("Too many sync wait commands"), including on Tile's own kernel-tail
    drain. Move excess waits onto standalone EventSemaphore instructions
    inserted just before the owner — the same-engine sequencer executes them
    in order, which is semantically identical."""
    import concourse.mybir as mybir

    n = 0
    for fn in nc.m.functions:
        for blk in fn.blocks:
            out = []
            for inst in list(blk.instructions):
                si = inst.sync_info
                waits = list(si.on_wait) if si is not None else []
                if len(waits) > max_waits:
                    for w in waits[max_waits:]:
                        n += 1
                        out.append(
                            mybir.InstEventSemaphore(
                                name=f"WSPLIT-{n}",
                                engine=inst.engine,
                                ins=[],
                                outs=[],
                                sync_info=mybir.SyncInfo(on_wait=[w], on_update=[]),
                            )
                        )
                    inst.sync_info = mybir.SyncInfo(
                        on_wait=waits[:max_waits], on_update=list(si.on_update)
                    )
                out.append(inst)
            blk.instructions = out
    return n


def build_decoder_program(
    B,
    frame,
    basis,
    spk,
    Lk,
    fb=512,
    split_waits=True,
    repeat=1,
    stage_bf16=False,
    loads_only=False,
):
    """Build the per-core Bass program. All shapes parameterized so the same
    builder can be validated in CoreSim at small sizes."""
    import concourse.bass as bass
    import concourse.mybir as mybir
    import concourse.tile as tile
    from concourse.bass import ds
    from contextlib import ExitStack

    f32 = mybir.dt.float32
    step = Lk // 2
    nseg = frame + 1
    out_len = (frame - 1) * step + Lk
    assert out_len == nseg * step
    dbl = basis * spk
    KC = basis // 128  # c-chunks per speaker
    NCH = dbl // 128  # c2 chunks total
    nblocks = (frame + fb - 1) // fb
    nsub_max = fb // 128

    nc = bass.Bass()
    # host concatenates inputs and estmask along the channel dim:
    # cat[b, f, 0:basis] = inputs, cat[b, f, basis:basis+dbl] = estmask
    cw = basis + dbl
    cat_d = nc.dram_tensor("cat", [B, frame, cw], f32, kind="ExternalInput")
    w_d = nc.dram_tensor(
        "w", [basis, Lk], mybir.dt.bfloat16 if stage_bf16 else f32, kind="ExternalInput"
    )
    ident_d = nc.dram_tensor("ident", [128, 128], f32, kind="ExternalInput")
    out_d = nc.dram_tensor("out", [B, spk, out_len], f32, kind="ExternalOutput")

    with ExitStack() as ctx:
        tc = ctx.enter_context(tile.TileContext(nc))
        singles = ctx.enter_context(tc.tile_pool(name="singles", bufs=1))
        mk_pool = ctx.enter_context(tc.tile_pool(name="mk", bufs=2))
        xx_pool = ctx.enter_context(tc.tile_pool(name="xx", bufs=2))
        xxt_pool = ctx.enter_context(tc.tile_pool(name="xxt", bufs=3))
        yrow_pool = ctx.enter_context(tc.tile_pool(name="yrow", bufs=1))
        st_pool = ctx.enter_context(tc.tile_pool(name="st", bufs=4))
        z_pool = ctx.enter_context(tc.tile_pool(name="z", bufs=4))
        tp_psum = ctx.enter_context(tc.tile_pool(name="tp_psum", bufs=2, space="PSUM"))
        yy_psum = ctx.enter_context(tc.tile_pool(name="yy_psum", bufs=4, space="PSUM"))
        zt_psum = ctx.enter_context(tc.tile_pool(name="zt_psum", bufs=2, space="PSUM"))

        w_sb = singles.tile([128, KC, Lk], mybir.dt.bfloat16 if stage_bf16 else f32)
        nc.sync.dma_start(out=w_sb, in_=w_d[:].rearrange("(k p) l -> p k l", p=128))
        ident = singles.tile([128, 128], f32)
        nc.sync.dma_start(out=ident, in_=ident_d[:])

        for b in [b for _ in range(repeat) for b in range(B)]:
            # zb[s][j, k] = y_s[k-1, j+step]  (zero at k = 0)
            zb = [
                yrow_pool.tile([step, nseg], f32, tag=f"zb{s}", name=f"zb{s}")
                for s in range(spk)
            ]
            for s in range(spk):
                nc.vector.memset(zb[s][:, 0:1], 0.0)
            for ib in range(nblocks):
                f0 = ib * fb
                Fb = min(fb, frame - f0)
                nsub = (Fb + 127) // 128
                cat_t = mk_pool.tile([128, nsub_max, cw], f32, tag="cat_t")
                # alternate the two HWDGE rings (SP / ACT) so big loads overlap
                # across queue-switch gaps
                ldeng = nc.sync if ib % 2 == 0 else nc.scalar
                if Fb == nsub_max * 128:
                    ldeng.dma_start(
                        out=cat_t,
                        in_=cat_d[b, f0 : f0 + Fb, :].rearrange("(a p) c -> p a c", p=128),
                    )
                else:
                    for a in range(nsub):
                        ps = min(128, Fb - a * 128)
                        ldeng.dma_start(
                            out=cat_t[:ps, a, :],
                            in_=cat_d[b, f0 + a * 128 : f0 + a * 128 + ps, :],
                        )
                if loads_only:
                    continue
                xdt = mybir.dt.bfloat16 if stage_bf16 else f32
                xx_t = xx_pool.tile([128, nsub_max, dbl], xdt, tag="xx_t")
                if stage_bf16 and Fb != nsub_max * 128:
                    # zero-pad ragged frames so full-size xbar transposes are
                    # legal (the mask-multiply then overwrites the valid rows)
                    nc.vector.memset(xx_t[:, nsub - 1, :], 0.0)
                for s in range(spk):
                    for a in range(nsub):
                        ps = min(128, Fb - a * 128)
                        mk_r = cat_t[:ps, a, basis:].rearrange(
                            "p (c two) -> p two c", two=2
                        )
                        # GPSIMD takes one subtile per speaker (2-input ops run
                        # ~2x slower there, but the Pool engine is idle)
                        eng = nc.gpsimd if a == 3 else nc.vector
                        eng.tensor_mul(
                            xx_t[:ps, a, ds(s * basis, basis)],
                            cat_t[:ps, a, 0:basis],
                            mk_r[:, s, :],
                        )
                xxT_t = xxt_pool.tile([128, NCH, fb], xdt, tag="xxT_t")
                if stage_bf16:
                    # xbar DMA-transpose (2-byte dtype): skips PE+PSUM+ACT entirely
                    for k in range(NCH):
                        for a in range(nsub):
                            nc.sync.dma_start(
                                out=xxT_t[:, k, ds(a * 128, 128)],
                                in_=xx_t[:, a, ds(k * 128, 128)],
                                transpose=True,
                            )
                else:
                    for k in range(NCH):
                        ps_t = tp_psum.tile([128, fb], f32, tag="ps_t")
                        for a in range(nsub):
                            ps = min(128, Fb - a * 128)
                            nc.tensor.transpose(
                                ps_t[:, ds(a * 128, ps)],
                                xx_t[:ps, a, ds(k * 128, 128)],
                                ident[:ps, :ps],
                            )
                        nc.scalar.copy(out=xxT_t[:, k, :Fb], in_=ps_t[:, :Fb])
                for s in range(spk):
                    yy_t = yy_psum.tile([Lk, fb], f32, tag="yy_t")
                    for kc in range(KC):
                        k = s * KC + kc
                        nc.tensor.matmul(
                            yy_t[:, :Fb],
                            w_sb[:, kc, :],
                            xxT_t[:, k, :Fb],
                            start=(kc == 0),
                            stop=(kc == KC - 1),
                        )
                    st_t = st_pool.tile([Lk, fb], f32, tag="st_t")
                    nc.scalar.copy(out=st_t[:, :Fb], in_=yy_t[:, :Fb])
                    # partition-shift the high taps into the row buffer
                    # (gpsimd queue: the Pool engine is otherwise idle)
                    nc.gpsimd.dma_start(
                        out=zb[s][:, f0 + 1 : f0 + 1 + Fb],
                        in_=st_t[step:Lk, :Fb],
                    )
                    z_t = z_pool.tile([step, fb], f32, tag="z_t")
                    nc.vector.tensor_add(
                        z_t[:, :Fb], st_t[0:step, :Fb], zb[s][:, f0 : f0 + Fb]
                    )
                    # PE-transpose z so the DRAM store writes 32B-contiguous
                    # runs (a [8, Fb] j-on-partition store would emit one 4B
                    # descriptor per element - a descriptor bomb).
                    zsub = (Fb + 127) // 128
                    zt_ps = zt_psum.tile([128, nsub_max, step], f32, tag="zt_ps")
                    for a in range(zsub):
                        ps = min(128, Fb - a * 128)
                        nc.tensor.transpose(
                            zt_ps[:ps, a, :],
                            z_t[:, ds(a * 128, ps)],
                            ident[0:step, 0:step],
                        )
                    ztc_t = st_pool.tile([128, nsub_max, step], f32, tag="ztc_t")
                    if Fb == nsub_max * 128:
                        nc.scalar.copy(out=ztc_t, in_=zt_ps)
                    else:
                        for a in range(zsub):
                            ps = min(128, Fb - a * 128)
                            nc.scalar.copy(
                                out=ztc_t[:ps, a, :], in_=zt_ps[:ps, a, :]
                            )
                    seg0 = f0 * step
                    if Fb == nsub_max * 128:
                        nc.scalar.dma_start(
                            out=out_d[b, s, seg0 : seg0 + Fb * step].rearrange(
                                "(a p j) -> p a j", p=128, j=step
                            ),
                            in_=ztc_t,
                        )
                    else:
                        for a in range(zsub):
                            ps = min(128, Fb - a * 128)
                            nc.scalar.dma_start(
                                out=out_d[
                                    b,
                                    s,
                                    seg0 + a * 128 * step : seg0 + (a * 128 + ps) * step,
                                ].rearrange("(p j) -> p j", j=step),
                                in_=ztc_t[:ps, a, :],
                            )
            for s in range(spk):
                # tail segment k = frame: z = y[frame-1, j+step] only
                nc.scalar.dma_start(
                    out=out_d[b, s, :].rearrange("(k j) -> j k", j=step)[
                        :, nseg - 1 : nseg
                    ],
                    in_=zb[s][:, nseg - 1 : nseg],
                )
    if split_waits:
        _split_excess_waits(nc)
    return nc


_PROGRAM_CACHE = {}

# bf16 staging of the transposed activations: halves transpose cost by moving
# it from PE+PSUM+ACT onto the DMA xbar. Leaves the mask-multiply in fp32.
STAGE_BF16 = False


def _get_program():
    key = (B_PER_CORE, FRAME, BASIS, SPK, L)
    ck = key + (STAGE_BF16,)
    if ck not in _PROGRAM_CACHE:
        _PROGRAM_CACHE[ck] = build_decoder_program(*key, stage_bf16=STAGE_BF16)
    return _PROGRAM_CACHE[ck]


def prepare_in_maps(inputs, estmask, W):
    """Shard the full inputs into per-core input maps."""
    inputs = np.asarray(inputs, dtype=np.float32)
    estmask = np.asarray(estmask, dtype=np.float32)
    if STAGE_BF16:
        import ml_dtypes

        W = np.ascontiguousarray(np.asarray(W).astype(ml_dtypes.bfloat16))
    else:
        W = np.ascontiguousarray(np.asarray(W, dtype=np.float32))
    cat = np.concatenate([inputs, estmask.reshape(BS, FRAME, BASIS * SPK)], axis=2)
    ident = np.eye(128, dtype=np.float32)

    in_maps = []
    for c in range(N_CORES):
        b0 = c * B_PER_CORE
        in_maps.append(
            {
                "cat": cat[b0 : b0 + B_PER_CORE],
                "w": W,
                "ident": ident,
            }
        )
    return in_maps


def run(inputs, estmask, W, trace=False):
    """Shard across 8 cores, run SPMD, gather. Returns (out, BassKernelResults)."""
    from concourse.bass_utils import run_bass_kernel_spmd

    nc = _get_program()
    in_maps = prepare_in_maps(inputs, estmask, W)
    res = run_bass_kernel_spmd(nc, in_maps, core_ids=list(range(N_CORES)), trace=trace)
    out = np.empty((BS, SPK, OUT_LEN), dtype=np.float32)
    for c in range(N_CORES):
        out[c * B_PER_CORE : (c + 1) * B_PER_CORE] = res.results[c]["out"]
    return out, res


def kernel(inputs, estmask, W, kernel_size_enc=None, speech_length=None):
    out, _ = run(inputs, estmask, W, trace=False)
    return out

